# revision 4
# baseline (speedup 1.0000x reference)
"""Trainium2 Bass kernel for nn_DiagScanModule: anti-diagonal scan reorder.

For each (b, c) plane of x (8, 64, 512, 512), produce two length-262144
sequences: the plane's elements in 'rd' anti-diagonal order (d = i+j,
i ascending within a diagonal) and 'ld' order (d = j-i+511, i ascending).

Strategy (per core; batch-sharded across 8 cores):
  The elements of rd-diagonal d live at flat offsets 511*i + d (stride 511);
  ld-diagonal d at 513*i + d - 511.  Define the sheared matrix
  W[i, d] = x_flat[stride*i + d + doff]: column d of W is diagonal d.
  1. DMA-load W tiles (contiguous 512B bursts per row, ~1.25x read amp),
     32 channels per DMA.
  2. PE-transpose 128x128 tiles (f32 identity matmul) -> V[d, i] in PSUM:
     each diagonal is now contiguous along the free axis.
  3. Copy PSUM -> SBUF V tile laid out [d_partition, channel, i_slot].
  4. One output DMA per (diagonal, 32-channel group): contiguous
     variable-length segment to its exact offset in y.
The index maps are compile-time constants of H=W=512 (reference's
_diag_maps), so all offsets/lengths are hardcoded into the access patterns
and the index-map inputs are not read on device.
"""

import os

import numpy as np

import concourse.bass as bass
import concourse.mybir as mybir
from concourse import masks
from concourse.tile import TileContext
from concourse.bass_utils import run_bass_kernel_spmd

# ---------------------------------------------------------------- geometry

B, C, H, W = 8, 64, 512, 512
HW = H * W            # 262144
ND = H + W - 1        # 1023 diagonals
PAD = 512             # front pad (elements) so ld's earliest reads stay in-bounds
CG = 32               # channels per group (per output DMA)
DBLK = 128            # diagonals per block (= PE transpose width)

F32 = mybir.dt.float32


def _geom(kind):
    d = np.arange(ND)
    ln = 512 - np.abs(511 - d)
    if kind == "rd":
        stride, doff = 511, 0
        s = np.maximum(0, d - 511)
    else:
        stride, doff = 513, -511
        s = np.maximum(0, 511 - d)
    off = np.concatenate([[0], np.cumsum(ln)[:-1]])
    return stride, doff, s.astype(int), ln.astype(int), off.astype(int)


def _blocks():
    return [(d0, min(DBLK, ND - d0)) for d0 in range(0, ND, DBLK)]


# ---------------------------------------------------------------- tile patch

def _patch_tile_drain():
    """walrus in this container rejects the TileContext exit drain when it
    carries semaphore waits ('Too many sync wait commands').  Emit the waits
    as individual NoOps instead and keep drains waitless."""
    import concourse.tile as tile_mod
    from concourse.vector_clock import ScopedClock

    if getattr(tile_mod.TileContext, "_diag_drain_patched", False):
        return

    def _drain_and_barrier(self, tick_clock, wait_clock):
        nc = self.nc
        drain_inst = nc.sync.drain(fusable=False)
        wait_clock.add_sem_waits(
            drain_inst.ins, ScopedClock({None: tick_clock.global_clock})
        )
        si = drain_inst.ins.sync_info
        waits = list(si.on_wait) if si is not None else []
        if waits:
            drain_inst.ins.sync_info = mybir.SyncInfo(on_wait=[], on_update=[])
            for w in waits:
                ni = nc.sync.nop()
                ni.ins.sync_info = mybir.SyncInfo(on_wait=[w], on_update=[])
            nc.sync.drain(fusable=False)

        nc.all_engine_barrier()
        assert self.sems is not None
        popped = nc._tile_sem_poison_stack.pop()
        assert popped is self._sem_poison
        nc.clear_and_free_semaphores(list(self.sems.allocated().values()))
        nc.all_engine_barrier()

    tile_mod.TileContext._drain_and_barrier = _drain_and_barrier
    tile_mod.TileContext._diag_drain_patched = True


def _split_multi_waits(nc, max_waits=1):
    """walrus here rejects instructions carrying more than one semaphore
    wait ('Too many sync wait commands').  Hoist excess waits onto NoOps
    inserted just before the instruction on the same engine — the engine
    blocks on each in program order, which preserves the sync semantics."""
    k = 0
    for fn in nc.m.functions:
        for bb in fn.blocks:
            new = []
            dirty = False
            for inst in bb.instructions:
                si = inst.sync_info
                waits = list(si.on_wait) if si is not None else []
                if len(waits) > max_waits:
                    for w in waits[:-max_waits]:
                        nop = mybir.InstNoOp(name=f"WSPLIT-{k}", ins=[], outs=[])
                        k += 1
                        nop.engine = inst.engine
                        nop.sync_info = mybir.SyncInfo(on_wait=[w], on_update=[])
                        new.append(nop)
                    inst.sync_info = mybir.SyncInfo(
                        on_wait=waits[-max_waits:], on_update=list(si.on_update)
                    )
                    dirty = True
                new.append(inst)
            if dirty:
                bb.instructions = new


# ---------------------------------------------------------------- kernel build

def _build_nc():
    _patch_tile_drain()
    nc = bass.Bass()
    x_t = nc.dram_tensor("x", [PAD + C * HW], F32, kind="ExternalInput")
    y_t = {
        "rd": nc.dram_tensor("y_rd", [C * HW], F32, kind="ExternalOutput"),
        "ld": nc.dram_tensor("y_ld", [C * HW], F32, kind="ExternalOutput"),
    }

    out_engines = None  # set inside context

    with TileContext(nc) as tc:
        with (
            tc.tile_pool(name="const", bufs=1) as cpool,
            tc.tile_pool(name="w", bufs=2) as wpool,
            tc.tile_pool(name="v", bufs=2) as vpool,
            tc.tile_pool(name="ps", bufs=8, space="PSUM") as ppool,
        ):
            ident = cpool.tile([128, 128], F32, tag="ident")
            masks.make_identity(nc, ident[:])

            out_engines = [nc.sync, nc.scalar, nc.gpsimd]
            oe = 0

            for kind in ("rd", "ld"):
                stride, doff, s, ln, off = _geom(kind)
                for cg in range(C // CG):
                    cbase = cg * CG
                    for (d0, D) in _blocks():
                        ds = np.arange(d0, d0 + D)
                        i_lo = int(np.min(s[ds]))
                        i_hi = int(np.max(s[ds] + ln[ds]))
                        span = i_hi - i_lo
                        nchunk = (span + 127) // 128

                        V = vpool.tile([128, CG, 512], F32, tag="V")
                        for k in range(nchunk):
                            r0 = i_lo + 128 * k
                            R = min(128, i_hi - r0)
                            Wt = wpool.tile([128, CG, 128], F32, tag="W")
                            src = bass.AP(
                                x_t,
                                PAD + cbase * HW + stride * r0 + d0 + doff,
                                [[stride, R], [HW, CG], [1, D]],
                            )
                            nc.sync.dma_start(out=Wt[:R, :, :D], in_=src)
                            for c in range(CG):
                                P = ppool.tile([128, 128], F32, tag="P")
                                nc.tensor.transpose(
                                    P[:D, :R], Wt[:R, c, :D], ident[:R, :R]
                                )
                                nc.vector.tensor_copy(
                                    V[:D, c, 128 * k : 128 * k + R], P[:D, :R]
                                )
                        for dd in range(D):
                            d = d0 + dd
                            a = int(s[d]) - i_lo
                            L = int(ln[d])
                            dst = bass.AP(
                                y_t[kind],
                                cbase * HW + int(off[d]),
                                [[HW, CG], [1, L]],
                            )
                            eng = out_engines[oe % len(out_engines)]
                            oe += 1
                            eng.dma_start(out=dst, in_=V[dd : dd + 1, :, a : a + L])
    _split_multi_waits(nc)
    return nc


_NC_CACHE = None
LAST_RESULTS = None


def kernel(x, rd_index_map=None, ld_index_map=None):
    """Full-input entry point: x (8, 64, 512, 512) f32 -> (y_rd, y_ld),
    each (8, 64, 262144) f32.  Index maps are deterministic functions of
    H=W=512 (see reference _diag_maps) and are baked into the kernel's
    access patterns, so they are not read here."""
    global _NC_CACHE, LAST_RESULTS
    x = np.ascontiguousarray(np.asarray(x), dtype=np.float32)
    assert x.shape == (B, C, H, W), x.shape

    if _NC_CACHE is None:
        _NC_CACHE = _build_nc()
    nc = _NC_CACHE

    in_maps = []
    for b in range(B):
        xb = np.empty(PAD + C * HW, np.float32)
        xb[:PAD] = 0.0
        xb[PAD:] = x[b].reshape(-1)
        in_maps.append({"x": xb})

    trace = bool(int(os.environ.get("DIAG_TRACE", "0")))
    res = run_bass_kernel_spmd(
        nc,
        in_maps,
        core_ids=list(range(B)),
        trace=trace,
    )
    LAST_RESULTS = res

    y_rd = np.empty((B, C, HW), np.float32)
    y_ld = np.empty((B, C, HW), np.float32)
    for b in range(B):
        y_rd[b] = res.results[b]["y_rd"].reshape(C, HW)
        y_ld[b] = res.results[b]["y_ld"].reshape(C, HW)
    return (y_rd, y_ld)
